# revision 30
# baseline (speedup 1.0000x reference)
"""Trainium2 Bass kernel for nn_Decoder_91190745629213 (RelGCN decoder).

Self-contained: hardcodes all shapes/sharding. Runs SPMD on 8 NeuronCores via
concourse (Bass/Tile) + run_bass_kernel_spmd.

v3 dataflow:
  * Layer 0 edge-source rows are HOST-pregathered (indices are static), so the
    kernel streams them contiguously (HWDGE) - no SWDGE descriptor emission,
    no x0 phase, no layer-0 AllGather.
  * Node->block assignment is balanced by 5-dim bin packing so every
    (block, relation) group fits TPB=9 edge tiles (1152 slots) instead of 10.
  * Everything in the GNN runs bf16 (PE 1 cycle/row, half the DMA bytes).
  * Layer 1: AllGather x1 (bf16), SWDGE dma_gather per (block, relation),
    one-hot scatter matmuls, W matmuls, tanh -> x2 (bf16, zero-padded row for
    ownership masking).
  * Final phase: transposed dma_gather builds frame-embedding columns directly
    (no PE transposes); per-core-masked logits + gold rows AllReduced.
"""

import os
import sys

sys.path.insert(0, "/opt/trn_rl_repo")

import numpy as np

# ---------------------------------------------------------------- config

P = 128
D = 512
KC = D // P          # 4 feature chunks
R = 5
N = 11201
B = 32
FRAMES = 1200
NROLE = 10001
E = 100000
NCORE = 8
BPC = 11             # blocks per core
NBLK = NCORE * BPC   # 88
NPAD = NBLK * P      # 11264
NLOC = BPC * P       # 1408
ZROW = NLOC          # zeroed row in padded x2 (ownership masking)
X2ROWS = NLOC + P    # 1536
SEG0B = 6            # blocks per core in AllGather segment 0
SEG0 = SEG0B * P     # 768 rows

SPAN_K = 2048
SPAN_SL = SPAN_K // NCORE  # 256 hidden features per core

FR_PAD = 1280        # frames padded for final gather (2 x 640)
FRH = FR_PAD // 2

_nc_cache = {}


# ---------------------------------------------------------------- program


def build_program(tpb):
    import concourse.mybir as mybir
    import concourse.tile as tile
    from concourse import bacc
    from concourse.masks import make_identity

    f32 = mybir.dt.float32
    bf16 = mybir.dt.bfloat16
    i16 = mybir.dt.int16
    AF = mybir.ActivationFunctionType
    ALU = mybir.AluOpType

    cap = tpb * P
    h0 = (tpb + 1) // 2          # tiles in first gather half
    h1 = tpb - h0
    assert h0 * P <= 1024 and h1 * P <= 1024

    nc = bacc.Bacc(None, target_bir_lowering=False, debug=False,
                   num_devices=NCORE)

    NT = BPC * R * tpb           # edge tiles per core
    NIDX = BPC * R * cap         # gather indices per core (layer 1)

    # ---- external inputs
    gidx = nc.declare_dram_parameter("gidx", [P, NIDX // 16], i16, isOutput=False)
    xg0 = nc.declare_dram_parameter("xg0", [P, NT, D], bf16, isOutput=False)
    rows_sh = nc.declare_dram_parameter("rows_sh", [P, NT], bf16, isOutput=False)
    vals_in = nc.declare_dram_parameter("vals_in", [P, NT], bf16, isOutput=False)
    iota_in = nc.declare_dram_parameter("iota_in", [P, cap], bf16, isOutput=False)
    wcat0 = nc.declare_dram_parameter("wcat0", [P, R * KC, D], bf16, isOutput=False)
    wcat1 = nc.declare_dram_parameter("wcat1", [P, R * KC, D], bf16, isOutput=False)
    tsT = nc.declare_dram_parameter("tsT", [P, SPAN_K // P, B], f32, isOutput=False)
    w1s = nc.declare_dram_parameter("w1s", [P, SPAN_K // P, SPAN_SL // P, P], f32, isOutput=False)
    b1s = nc.declare_dram_parameter("b1s", [P, SPAN_SL // P], f32, isOutput=False)
    w2s = nc.declare_dram_parameter("w2s", [P, SPAN_SL // P, KC, P], f32, isOutput=False)
    b2f = nc.declare_dram_parameter("b2f", [P, KC], f32, isOutput=False)
    fpw1 = nc.declare_dram_parameter("fpw1", [P, KC, KC, P], f32, isOutput=False)
    fpb1 = nc.declare_dram_parameter("fpb1", [P, KC], f32, isOutput=False)
    fpw2 = nc.declare_dram_parameter("fpw2", [P, KC, KC, P], f32, isOutput=False)
    fpb2 = nc.declare_dram_parameter("fpb2", [P, KC], f32, isOutput=False)
    fridx = nc.declare_dram_parameter("fridx", [P, FR_PAD // 16], i16, isOutput=False)
    goldidx = nc.declare_dram_parameter("goldidx", [P, P // 16], i16, isOutput=False)

    out = nc.declare_dram_parameter("out", [B, FRAMES + D], f32, isOutput=True)

    # ---- internal DRAM
    x1a = nc.dram_tensor("x1a", [SEG0, D], bf16)
    x1b = nc.dram_tensor("x1b", [NLOC - SEG0, D], bf16)
    x_full = nc.dram_tensor("x_full", [NPAD, D], bf16, addr_space="Shared")
    x2_sl = nc.dram_tensor("x2_sl", [X2ROWS, D], bf16)
    tn_part = nc.dram_tensor("tn_part", [P, KC, B], f32)
    tn_red = nc.dram_tensor("tn_red", [P, KC, B], f32, addr_space="Shared")
    pg_part = nc.dram_tensor("pg_part", [B, FRAMES + D], f32)
    pg_red = nc.dram_tensor("pg_red", [B, FRAMES + D], f32, addr_space="Shared")

    groups = [list(range(NCORE))]

    with tile.TileContext(nc) as tc:
        with (
            tc.tile_pool(name="const", bufs=1) as cpool,
            tc.tile_pool(name="work", bufs=2) as pool,
            tc.tile_pool(name="gath", bufs=2) as gpool,
            tc.tile_pool(name="psA", bufs=2, space="PSUM") as psA,
            tc.tile_pool(name="psB", bufs=2, space="PSUM") as psB,
            tc.tile_pool(name="psS", bufs=1, space="PSUM") as psS,
        ):
            # ---------- constants into SBUF
            ident = cpool.tile([P, P], bf16)
            make_identity(nc, ident[:])
            idx_sb = cpool.tile([P, NIDX // 16], i16)
            nc.sync.dma_start(out=idx_sb[:], in_=gidx[:])
            rows_sb = cpool.tile([P, NT], bf16)
            nc.sync.dma_start(out=rows_sb[:], in_=rows_sh[:])
            vals_sb = cpool.tile([P, NT], bf16)
            nc.sync.dma_start(out=vals_sb[:], in_=vals_in[:])
            iota_sb = cpool.tile([P, cap], bf16)
            nc.sync.dma_start(out=iota_sb[:], in_=iota_in[:])
            fridx_sb = cpool.tile([P, FR_PAD // 16], i16)
            nc.sync.dma_start(out=fridx_sb[:], in_=fridx[:])
            goldidx_sb = cpool.tile([P, P // 16], i16)
            nc.sync.dma_start(out=goldidx_sb[:], in_=goldidx[:])
            wc0 = cpool.tile([P, R * KC, D], bf16)
            nc.sync.dma_start(out=wc0[:], in_=wcat0[:])
            wc1 = cpool.tile([P, R * KC, D], bf16)
            nc.sync.dma_start(out=wc1[:], in_=wcat1[:])

            tsT_sb = cpool.tile([P, SPAN_K // P, B], f32)
            nc.sync.dma_start(out=tsT_sb[:], in_=tsT[:])
            b1s_sb = cpool.tile([P, SPAN_SL // P], f32)
            nc.sync.dma_start(out=b1s_sb[:], in_=b1s[:])
            b2f_sb = cpool.tile([P, KC], f32)
            nc.sync.dma_start(out=b2f_sb[:], in_=b2f[:])
            fpb1_sb = cpool.tile([P, KC], f32)
            nc.sync.dma_start(out=fpb1_sb[:], in_=fpb1[:])
            fpb2_sb = cpool.tile([P, KC], f32)
            nc.sync.dma_start(out=fpb2_sb[:], in_=fpb2[:])

            # zero pad rows of x2 (row ZROW used to mask non-owned gathers)
            zt = cpool.tile([P, D], bf16)
            nc.vector.memset(zt[:], 0.0)
            nc.sync.dma_start(out=x2_sl[NLOC:X2ROWS, :], in_=zt[:])

            # ---------- span MLP (K-sharded; independent of GNN)
            h1T_sb = pool.tile([P, SPAN_SL // P, B], f32, tag="h1T")
            for mc in range(SPAN_SL // P):
                ph = psS.tile([P, B], f32, tag="sp")
                for kc in range(SPAN_K // P):
                    w1t = pool.tile([P, P], f32, tag="w1t")
                    nc.sync.dma_start(out=w1t[:], in_=w1s[:, kc, mc, :])
                    nc.tensor.matmul(out=ph[:], lhsT=w1t[:], rhs=tsT_sb[:, kc, :],
                                     start=(kc == 0), stop=(kc == SPAN_K // P - 1))
                nc.scalar.activation(out=h1T_sb[:, mc, :], in_=ph[:], func=AF.Relu,
                                     bias=b1s_sb[:, mc:mc + 1])
            tnp_sb = pool.tile([P, KC, B], f32, tag="tnp")
            for mc in range(KC):
                ph = psS.tile([P, B], f32, tag="sp")
                for kc in range(SPAN_SL // P):
                    w2t = pool.tile([P, P], f32, tag="w1t")
                    nc.sync.dma_start(out=w2t[:], in_=w2s[:, kc, mc, :])
                    nc.tensor.matmul(out=ph[:], lhsT=w2t[:],
                                     rhs=h1T_sb[:, kc, :],
                                     start=(kc == 0), stop=(kc == SPAN_SL // P - 1))
                nc.vector.tensor_copy(out=tnp_sb[:, mc, :], in_=ph[:])
            nc.sync.dma_start(out=tn_part[:], in_=tnp_sb[:])
            # dispatched first on the CC stream: absorbs the collective
            # cold-start in parallel with layer 0 (consumers come after L1).
            nc.gpsimd.collective_compute(
                "AllReduce", ALU.add, replica_groups=groups,
                ins=[tn_part[:]], outs=[tn_red[:]])

            # ---------- GNN layers
            for layer in range(2):
                wc = wc0 if layer == 0 else wc1
                if layer == 1:
                    # split AllGather: first 6 blocks of every core fire as
                    # soon as they are written, overlapping the layer-0 tail.
                    nc.gpsimd.collective_compute(
                        "AllGather", ALU.bypass, replica_groups=groups,
                        ins=[x1a[:]], outs=[x_full[0:NCORE * SEG0, :]])
                    nc.gpsimd.collective_compute(
                        "AllGather", ALU.bypass, replica_groups=groups,
                        ins=[x1b[:]], outs=[x_full[NCORE * SEG0:NPAD, :]])

                for b in range(BPC):
                    gT_sb = pool.tile([P, R * KC, P], bf16, tag="gT")
                    for r in range(R):
                        bat = b * R + r
                        t0 = bat * tpb
                        if layer == 0:
                            xg = gpool.tile([P, tpb, D], bf16, tag="xg0")
                            nc.sync.dma_start(out=xg[:],
                                              in_=xg0[:, t0:t0 + tpb, :])

                            def tsl(t, xg=xg):
                                return xg[:, t, :]
                        else:
                            i0 = bat * (cap // 16)
                            mb0 = gpool.tile([P, h0, D], bf16, tag="mbufa")
                            mb1 = gpool.tile([P, h1, D], bf16, tag="mbufb")
                            nc.gpsimd.dma_gather(
                                out_ap=mb0[:], in_ap=x_full[:],
                                idxs_ap=idx_sb[:, i0:i0 + h0 * P // 16],
                                num_idxs=h0 * P, num_idxs_reg=h0 * P,
                                elem_size=D, elem_step=D)
                            nc.gpsimd.dma_gather(
                                out_ap=mb1[:], in_ap=x_full[:],
                                idxs_ap=idx_sb[:, i0 + h0 * P // 16:i0 + cap // 16],
                                num_idxs=h1 * P, num_idxs_reg=h1 * P,
                                elem_size=D, elem_step=D)

                            def tsl(t, a=mb0, b=mb1):
                                if t < h0:
                                    return a[:, t, :]
                                return b[:, t - h0, :]
                        s_sb = pool.tile([P, cap], bf16, tag="s")
                        rows_b = rows_sb[:, t0:t0 + tpb].unsqueeze(2).broadcast_to([P, tpb, P])
                        nc.vector.tensor_tensor(
                            out=s_sb[:].rearrange("p (t w) -> p t w", w=P),
                            in0=rows_b,
                            in1=iota_sb[:].rearrange("p (t w) -> p t w", w=P),
                            op=ALU.is_equal)
                        vals_b = vals_sb[:, t0:t0 + tpb].unsqueeze(2).broadcast_to([P, tpb, P])
                        nc.vector.tensor_tensor(
                            out=s_sb[:].rearrange("p (t w) -> p t w", w=P),
                            in0=s_sb[:].rearrange("p (t w) -> p t w", w=P),
                            in1=vals_b,
                            op=ALU.mult)
                        pg = psA.tile([P, D], f32, tag="pg", bufs=3)
                        for t in range(tpb):
                            nc.tensor.matmul(
                                out=pg[:],
                                lhsT=s_sb[:, t * P:(t + 1) * P],
                                rhs=tsl(t),
                                start=(t == 0), stop=(t == tpb - 1))
                        g_sb = pool.tile([P, D], bf16, tag="g")
                        nc.scalar.activation(out=g_sb[:], in_=pg[:], func=AF.Copy)
                        ptr = psA.tile([P, D], bf16, tag="ptr")
                        for c in range(KC):
                            nc.tensor.transpose(out=ptr[:, c * P:(c + 1) * P],
                                                in_=g_sb[:, c * P:(c + 1) * P],
                                                identity=ident[:])
                        nc.scalar.activation(
                            out=gT_sb[:, r * KC:(r + 1) * KC, :],
                            in_=ptr[:].rearrange("p (c w) -> p c w", w=P),
                            func=AF.Copy)
                    po = psB.tile([P, D], f32, tag="po")
                    for j in range(R * KC):
                        nc.tensor.matmul(out=po[:], lhsT=gT_sb[:, j, :],
                                         rhs=wc[:, j, :],
                                         start=(j == 0), stop=(j == R * KC - 1))
                    xo = pool.tile([P, D], bf16, tag="xo")
                    nc.scalar.activation(out=xo[:], in_=po[:], func=AF.Tanh)
                    if layer == 0:
                        if b < SEG0B:
                            nc.sync.dma_start(out=x1a[b * P:(b + 1) * P, :], in_=xo[:])
                        else:
                            nc.sync.dma_start(
                                out=x1b[(b - SEG0B) * P:(b - SEG0B + 1) * P, :],
                                in_=xo[:])
                    else:
                        nc.sync.dma_start(out=x2_sl[b * P:(b + 1) * P, :], in_=xo[:])

            # ---------- span MLP tail (consumes tn_red; placed after the GNN
            # loops so no engine queue stalls on the AllReduce mid-layer-0)
            tnT_sb = pool.tile([P, KC, B], f32, tag="tnT")
            tnr_sb = pool.tile([P, KC, B], f32, tag="tnr")
            nc.sync.dma_start(out=tnr_sb[:], in_=tn_red[:])
            for mc in range(KC):
                nc.vector.tensor_scalar_add(out=tnT_sb[:, mc, :], in0=tnr_sb[:, mc, :],
                                            scalar1=b2f_sb[:, mc:mc + 1])
            h2T_sb = pool.tile([P, KC, B], f32, tag="h2T")
            for mc in range(KC):
                ph = psS.tile([P, B], f32, tag="sp")
                for kc in range(KC):
                    fp1t = pool.tile([P, P], f32, tag="w1t")
                    nc.sync.dma_start(out=fp1t[:], in_=fpw1[:, kc, mc, :])
                    nc.tensor.matmul(out=ph[:], lhsT=fp1t[:],
                                     rhs=tnT_sb[:, kc, :],
                                     start=(kc == 0), stop=(kc == KC - 1))
                nc.scalar.activation(out=h2T_sb[:, mc, :], in_=ph[:], func=AF.Relu,
                                     bias=fpb1_sb[:, mc:mc + 1])
            qTb_sb = pool.tile([P, KC, B], bf16, tag="qTb")
            for mc in range(KC):
                ph = psS.tile([P, B], f32, tag="sp")
                for kc in range(KC):
                    fp2t = pool.tile([P, P], f32, tag="w1t")
                    nc.sync.dma_start(out=fp2t[:], in_=fpw2[:, kc, mc, :])
                    nc.tensor.matmul(out=ph[:], lhsT=fp2t[:],
                                     rhs=h2T_sb[:, kc, :],
                                     start=(kc == 0), stop=(kc == KC - 1))
                nc.scalar.activation(out=qTb_sb[:, mc, :], in_=ph[:], func=AF.Tanh,
                                     bias=fpb2_sb[:, mc:mc + 1])

            # ---------- final (localized): pred cols for OWN frames + own gold
            frT0 = cpool.tile([P, KC, FRH], bf16, tag="frT0")
            frT1 = cpool.tile([P, KC, FRH], bf16, tag="frT1")
            frT = [frT0, frT1]
            for h in range(2):
                nc.gpsimd.dma_gather(
                    out_ap=frT[h][:],
                    in_ap=x2_sl[:],
                    idxs_ap=fridx_sb[:, h * (FRH // 16):(h + 1) * (FRH // 16)],
                    num_idxs=FRH, num_idxs_reg=FRH,
                    elem_size=D, transpose=True)
            pg_sb = cpool.tile([B, FRAMES + D], f32, tag="pgsb")
            chunks = [(0, 0, 512), (0, 512, 128), (1, 0, 512), (1, 512, 48)]
            lo = 0
            for (h, off, w) in chunks:
                pp = psB.tile([B, w], f32, tag="po")
                for c in range(KC):
                    nc.tensor.matmul(out=pp[:], lhsT=qTb_sb[:, c, :],
                                     rhs=frT[h][:, c, off:off + w],
                                     start=(c == 0), stop=(c == KC - 1))
                nc.vector.tensor_copy(out=pg_sb[:, lo:lo + w], in_=pp[:])
                lo += w
            assert lo == FRAMES
            gold_sb = pool.tile([P, 1, D], bf16, tag="goldb")
            nc.gpsimd.dma_gather(
                out_ap=gold_sb[:], in_ap=x2_sl[:], idxs_ap=goldidx_sb[:],
                num_idxs=P, num_idxs_reg=P, elem_size=D, elem_step=D)
            nc.vector.tensor_copy(out=pg_sb[:, FRAMES:], in_=gold_sb[:B, 0, :])
            nc.sync.dma_start(out=pg_part[:], in_=pg_sb[:])
            nc.gpsimd.collective_compute(
                "AllReduce", ALU.add, replica_groups=groups,
                ins=[pg_part[:]], outs=[pg_red[:]])
            nc.sync.dma_start(out=out[:], in_=pg_red[:])

    nc.compile()
    return nc


def get_program(tpb):
    if tpb not in _nc_cache:
        _nc_cache[tpb] = build_program(tpb)
    return _nc_cache[tpb]


# ---------------------------------------------------------------- host prep


def _bf16():
    import ml_dtypes
    return ml_dtypes.bfloat16


def _wrap_idx16(flat):
    a = np.asarray(flat, np.int16).reshape(-1, 16).T  # [16, n/16]
    return np.tile(a, (8, 1)).copy()


def _segpos(bin_id, i):
    """Global x_full position of slot i of bin `bin_id` under the split
    (two-segment) AllGather layout."""
    core, bw = divmod(bin_id, BPC)
    if bw < SEG0B:
        return core * SEG0 + bw * P + i
    return NCORE * SEG0 + core * (NLOC - SEG0) + (bw - SEG0B) * P + i


def _pack_nodes(adj_rows):
    """Balanced 5-dim bin packing: assign NPAD nodes to NBLK bins of P nodes
    so that per-(bin, relation) in-edge counts are as flat as possible.
    Returns (perm, pos_of, bin_of, row_of, cap)."""
    deg = np.zeros((NPAD, R), np.int64)
    for r in range(R):
        np.add.at(deg[:, r], adj_rows[r], 1)
    order = np.argsort(-deg.sum(1), kind="stable")
    bin_load = np.zeros((NBLK, R), np.int64)
    bin_cnt = np.zeros(NBLK, np.int64)
    assign = np.empty(NPAD, np.int64)
    for n in order:
        d = deg[n]
        cand = np.where(bin_cnt < P)[0]
        newmax = (bin_load[cand] + d).max(1)
        best = cand[np.lexsort((bin_load[cand].sum(1), newmax))[0]]
        assign[n] = best
        bin_load[best] += d
        bin_cnt[best] += 1
    maxload = int(bin_load.max())
    cap = ((maxload + P - 1) // P) * P
    cap = max(cap, 2 * P)
    perm = np.empty(NPAD, np.int64)
    pos_of = np.empty(NPAD, np.int64)
    bin_of = np.empty(NPAD, np.int64)
    row_of = np.empty(NPAD, np.int64)
    for bin_id in range(NBLK):
        nodes = np.where(assign == bin_id)[0]
        q = _segpos(bin_id, np.arange(P))
        perm[q] = nodes
        pos_of[nodes] = q
        bin_of[q] = bin_id
        row_of[q] = np.arange(P)
    return perm, pos_of, bin_of, row_of, cap


def preprocess(inputs):
    bf = _bf16()
    ts = np.ascontiguousarray(np.asarray(inputs["target_span"], np.float32))
    frame_emb = np.asarray(inputs["frame_emb"], np.float32)
    role_emb = np.asarray(inputs["role_emb"], np.float32)
    rel_W0 = np.asarray(inputs["rel_W0"], np.float32)
    rel_W1 = np.asarray(inputs["rel_W1"], np.float32)
    span_W1 = np.asarray(inputs["span_W1"], np.float32)
    span_b1 = np.asarray(inputs["span_b1"], np.float32)
    span_W2 = np.asarray(inputs["span_W2"], np.float32)
    span_b2 = np.asarray(inputs["span_b2"], np.float32)
    fp_W1 = np.asarray(inputs["fp_W1"], np.float32)
    fp_b1 = np.asarray(inputs["fp_b1"], np.float32)
    fp_W2 = np.asarray(inputs["fp_W2"], np.float32)
    fp_b2 = np.asarray(inputs["fp_b2"], np.float32)
    adj_vals = np.asarray(inputs["adj_vals"], np.float32)
    fe_ids = np.asarray(inputs["fe_ids"]).astype(np.int64)
    adj_rows = np.asarray(inputs["adj_rows"]).astype(np.int64)
    adj_cols = np.asarray(inputs["adj_cols"]).astype(np.int64)
    gold_frame_id = np.asarray(inputs["gold_frame_id"]).astype(np.int64)
    frame_list = np.asarray(inputs["frame_list"]).astype(np.int64)

    perm, pos_of, bin_of, row_of, cap = _pack_nodes(
        [adj_rows[r] for r in range(R)])
    tpb = cap // P
    core_of = bin_of // BPC
    loc_of = (bin_of % BPC) * P + row_of     # local x2 row per position

    g_idx = np.zeros((R, NBLK, cap), np.int64)
    g_rows = np.zeros((R, NBLK, cap), np.float32)
    g_vals = np.zeros((R, NBLK, cap), np.float32)
    for r in range(R):
        pos_r = pos_of[adj_rows[r]]
        blk = bin_of[pos_r]
        order = np.argsort(blk, kind="stable")
        blk_s = blk[order]
        counts = np.bincount(blk_s, minlength=NBLK)
        assert counts.max() <= cap
        starts = np.zeros(NBLK, np.int64)
        starts[1:] = np.cumsum(counts)[:-1]
        rank = np.arange(E) - starts[blk_s]
        dest = blk_s * cap + rank
        g_idx[r].flat[dest] = pos_of[adj_cols[r][order]]
        g_rows[r].flat[dest] = row_of[pos_r[order]].astype(np.float32)
        g_vals[r].flat[dest] = adj_vals[r][order]

    emb_cat = np.concatenate([frame_emb[:FRAMES], role_emb], axis=0)
    assert emb_cat.shape == (N, D)

    node_at = perm
    src_all = np.where(node_at < FRAMES, node_at,
                       np.where(node_at < N,
                                FRAMES + fe_ids[np.clip(node_at - FRAMES, 0, NROLE - 1)],
                                0))
    emb_src = emb_cat[src_all].astype(bf)   # [NPAD, D] layer-0 x by position

    wcat0 = rel_W0.reshape(R, KC, P, D).transpose(2, 0, 1, 3).reshape(P, R * KC, D)
    wcat1 = rel_W1.reshape(R, KC, P, D).transpose(2, 0, 1, 3).reshape(P, R * KC, D)
    tsT = ts.T.reshape(SPAN_K // P, P, B).transpose(1, 0, 2)
    fpw1 = fp_W1.reshape(KC, P, KC, P).transpose(1, 0, 2, 3)
    fpw2 = fp_W2.reshape(KC, P, KC, P).transpose(1, 0, 2, 3)
    b2f = span_b2.reshape(KC, P).T
    fpb1v = fp_b1.reshape(KC, P).T
    fpb2v = fp_b2.reshape(KC, P).T

    iota = np.tile(np.arange(P, dtype=np.float32), tpb)[None, :].repeat(P, axis=0)

    gold_label = frame_list[np.arange(B), gold_frame_id]
    gold_pos = pos_of[gold_label]
    fr_pos = np.zeros(FR_PAD, np.int64)
    fr_pos[:FRAMES] = pos_of[np.arange(FRAMES)]
    fr_pos[FRAMES:] = -1

    in_maps = []
    for k in range(NCORE):
        blo, bhi = k * BPC, (k + 1) * BPC
        ci = g_idx[:, blo:bhi].transpose(1, 0, 2)       # [BPC, R, cap]
        cr = g_rows[:, blo:bhi].transpose(1, 0, 2)
        cv = g_vals[:, blo:bhi].transpose(1, 0, 2)
        flat_idx = ci.reshape(-1)
        gidx_c = _wrap_idx16(flat_idx)
        xg0_c = emb_src[flat_idx].reshape(-1, P, D).transpose(1, 0, 2)
        rows_c = cr.reshape(-1, P).T.astype(bf)
        vals_c = cv.reshape(-1, P).T.astype(bf)
        sl = slice(k * SPAN_SL, (k + 1) * SPAN_SL)
        w1slice = span_W1[:, sl]
        w1s = w1slice.reshape(SPAN_K // P, P, SPAN_SL // P, P).transpose(1, 0, 2, 3)
        b1sv = span_b1[sl].reshape(SPAN_SL // P, P).T
        w2slice = span_W2[sl, :]
        w2s = w2slice.reshape(SPAN_SL // P, P, KC, P).transpose(1, 0, 2, 3)

        frp = np.clip(fr_pos, 0, NPAD - 1)
        own_fr = (fr_pos >= 0) & (core_of[frp] == k)
        fr_local = np.where(own_fr, loc_of[frp], ZROW)
        own_gold = core_of[gold_pos] == k
        gold_local = np.where(own_gold, loc_of[gold_pos], ZROW)

        in_maps.append(dict(
            gidx=np.ascontiguousarray(gidx_c),
            xg0=np.ascontiguousarray(xg0_c),
            rows_sh=np.ascontiguousarray(rows_c),
            vals_in=np.ascontiguousarray(vals_c),
            iota_in=iota.astype(bf),
            wcat0=np.ascontiguousarray(wcat0).astype(bf),
            wcat1=np.ascontiguousarray(wcat1).astype(bf),
            tsT=np.ascontiguousarray(tsT),
            w1s=np.ascontiguousarray(w1s),
            b1s=np.ascontiguousarray(b1sv),
            w2s=np.ascontiguousarray(w2s),
            b2f=np.ascontiguousarray(b2f),
            fpw1=np.ascontiguousarray(fpw1),
            fpb1=np.ascontiguousarray(fpb1v),
            fpw2=np.ascontiguousarray(fpw2),
            fpb2=np.ascontiguousarray(fpb2v),
            fridx=_wrap_idx16(fr_local),
            goldidx=_wrap_idx16(np.concatenate(
                [gold_local, np.full(P - B, ZROW, np.int64)])),
        ))
    return in_maps, tpb


def _maybe_enable_trace():
    import types
    import antenv
    if getattr(antenv, "axon_hooks", None) is not None:
        return
    mod = types.ModuleType("antenv.axon_hooks")
    state = {}
    mod.set_axon_ntff_profile_hook = lambda h: state.__setitem__("h", h)
    mod.get_axon_ntff_profile_hook = lambda: state.get("h")
    sys.modules["antenv.axon_hooks"] = mod
    antenv.axon_hooks = mod
    from trn_agent_boot.trn_boot import _ntff_profile_via_ctypes
    mod.set_axon_ntff_profile_hook(_ntff_profile_via_ctypes("/opt/axon/libaxon_pjrt.so"))


def kernel(**inputs):
    from concourse.bass_utils import run_bass_kernel_spmd

    trace = os.environ.get("KERNEL_TRACE", "0") == "1"
    if trace:
        _maybe_enable_trace()

    in_maps, tpb = preprocess(inputs)
    nc = get_program(tpb)
    kw = {}
    if trace:
        import tempfile
        kw = dict(trace=True, tmpdir=tempfile.mkdtemp(prefix="ktrace_"))
    res = run_bass_kernel_spmd(nc, in_maps, list(range(NCORE)), **kw)
    if trace:
        kernel.last_exec_time_ns = res.exec_time_ns
    return np.asarray(res.results[0]["out"], np.float32)


kernel.last_exec_time_ns = None


# revision 32
# speedup vs baseline: 1.1542x; 1.1542x over previous
"""Trainium2 Bass kernel for nn_Decoder_91190745629213 (RelGCN decoder).

Self-contained: hardcodes all shapes/sharding. Runs SPMD on 8 NeuronCores via
concourse (Bass/Tile) + run_bass_kernel_spmd.

v3 dataflow:
  * Layer 0 edge-source rows are HOST-pregathered (indices are static), so the
    kernel streams them contiguously (HWDGE) - no SWDGE descriptor emission,
    no x0 phase, no layer-0 AllGather.
  * Node->block assignment is balanced by 5-dim bin packing so every
    (block, relation) group fits TPB=9 edge tiles (1152 slots) instead of 10.
  * Everything in the GNN runs bf16 (PE 1 cycle/row, half the DMA bytes).
  * Layer 1: AllGather x1 (bf16), SWDGE dma_gather per (block, relation),
    one-hot scatter matmuls, W matmuls, tanh -> x2 (bf16, zero-padded row for
    ownership masking).
  * Final phase: transposed dma_gather builds frame-embedding columns directly
    (no PE transposes); per-core-masked logits + gold rows AllReduced.
"""

import os
import sys

sys.path.insert(0, "/opt/trn_rl_repo")

import numpy as np

# ---------------------------------------------------------------- config

P = 128
D = 512
KC = D // P          # 4 feature chunks
R = 5
N = 11201
B = 32
FRAMES = 1200
NROLE = 10001
E = 100000
NCORE = 8
BPC = 11             # blocks per core
NBLK = NCORE * BPC   # 88
NPAD = NBLK * P      # 11264
NLOC = BPC * P       # 1408
ZROW = NLOC          # zeroed row in padded x2 (ownership masking)
X2ROWS = NLOC + P    # 1536
SEG0B = 6            # blocks per core in AllGather segment 0
SEG0 = SEG0B * P     # 768 rows

SPAN_K = 2048
SPAN_SL = SPAN_K // NCORE  # 256 hidden features per core

FR_PAD = 1280        # frames padded for final gather (2 x 640)
FRH = FR_PAD // 2

_nc_cache = {}


# ---------------------------------------------------------------- program


def build_program(tpb):
    import concourse.mybir as mybir
    import concourse.tile as tile
    from concourse import bacc
    from concourse.masks import make_identity

    f32 = mybir.dt.float32
    bf16 = mybir.dt.bfloat16
    i16 = mybir.dt.int16
    AF = mybir.ActivationFunctionType
    ALU = mybir.AluOpType

    cap = tpb * P
    h0 = (tpb + 1) // 2          # tiles in first gather half
    h1 = tpb - h0
    assert h0 * P <= 1024 and h1 * P <= 1024

    nc = bacc.Bacc(None, target_bir_lowering=False, debug=False,
                   num_devices=NCORE)

    NT = BPC * R * tpb           # edge tiles per core
    NIDX = BPC * R * cap         # gather indices per core (layer 1)

    # ---- external inputs
    gidx = nc.declare_dram_parameter("gidx", [P, NIDX // 16], i16, isOutput=False)
    xg0 = nc.declare_dram_parameter("xg0", [P, NT, D], bf16, isOutput=False)
    rows_sh = nc.declare_dram_parameter("rows_sh", [P, NT], bf16, isOutput=False)
    vals_in = nc.declare_dram_parameter("vals_in", [P, NT], bf16, isOutput=False)
    iota_in = nc.declare_dram_parameter("iota_in", [P, cap], bf16, isOutput=False)
    wcat0 = nc.declare_dram_parameter("wcat0", [P, R * KC, D], bf16, isOutput=False)
    wcat1 = nc.declare_dram_parameter("wcat1", [P, R * KC, D], bf16, isOutput=False)
    tsT = nc.declare_dram_parameter("tsT", [P, SPAN_K // P, B], f32, isOutput=False)
    w1s = nc.declare_dram_parameter("w1s", [P, SPAN_K // P, SPAN_SL // P, P], f32, isOutput=False)
    b1s = nc.declare_dram_parameter("b1s", [P, SPAN_SL // P], f32, isOutput=False)
    w2s = nc.declare_dram_parameter("w2s", [P, SPAN_SL // P, KC, P], f32, isOutput=False)
    b2f = nc.declare_dram_parameter("b2f", [P, KC], f32, isOutput=False)
    fpw1 = nc.declare_dram_parameter("fpw1", [P, KC, KC, P], f32, isOutput=False)
    fpb1 = nc.declare_dram_parameter("fpb1", [P, KC], f32, isOutput=False)
    fpw2 = nc.declare_dram_parameter("fpw2", [P, KC, KC, P], f32, isOutput=False)
    fpb2 = nc.declare_dram_parameter("fpb2", [P, KC], f32, isOutput=False)
    fridx = nc.declare_dram_parameter("fridx", [P, FR_PAD // 16], i16, isOutput=False)
    goldidx = nc.declare_dram_parameter("goldidx", [P, P // 16], i16, isOutput=False)

    out = nc.declare_dram_parameter("out", [B, FRAMES + D], f32, isOutput=True)

    # ---- internal DRAM
    x1_slice = nc.dram_tensor("x1_slice", [NLOC, D], bf16)
    x_full = nc.dram_tensor("x_full", [NPAD, D], bf16, addr_space="Shared")
    x2_sl = nc.dram_tensor("x2_sl", [X2ROWS, D], bf16)
    tn_part = nc.dram_tensor("tn_part", [P, KC, B], f32)
    tn_red = nc.dram_tensor("tn_red", [P, KC, B], f32, addr_space="Shared")
    pg_part = nc.dram_tensor("pg_part", [B, FRAMES + D], f32)
    pg_red = nc.dram_tensor("pg_red", [B, FRAMES + D], f32, addr_space="Shared")

    groups = [list(range(NCORE))]

    with tile.TileContext(nc) as tc:
        with (
            tc.tile_pool(name="const", bufs=1) as cpool,
            tc.tile_pool(name="work", bufs=2) as pool,
            tc.tile_pool(name="gath", bufs=2) as gpool,
            tc.tile_pool(name="psA", bufs=2, space="PSUM") as psA,
            tc.tile_pool(name="psB", bufs=2, space="PSUM") as psB,
            tc.tile_pool(name="psS", bufs=1, space="PSUM") as psS,
        ):
            # ---------- constants into SBUF
            ident = cpool.tile([P, P], bf16)
            make_identity(nc, ident[:])
            idx_sb = cpool.tile([P, NIDX // 16], i16)
            nc.sync.dma_start(out=idx_sb[:], in_=gidx[:])
            rows_sb = cpool.tile([P, NT], bf16)
            nc.sync.dma_start(out=rows_sb[:], in_=rows_sh[:])
            vals_sb = cpool.tile([P, NT], bf16)
            nc.sync.dma_start(out=vals_sb[:], in_=vals_in[:])
            iota_sb = cpool.tile([P, cap], bf16)
            nc.sync.dma_start(out=iota_sb[:], in_=iota_in[:])
            fridx_sb = cpool.tile([P, FR_PAD // 16], i16)
            nc.sync.dma_start(out=fridx_sb[:], in_=fridx[:])
            goldidx_sb = cpool.tile([P, P // 16], i16)
            nc.sync.dma_start(out=goldidx_sb[:], in_=goldidx[:])
            wc0 = cpool.tile([P, R * KC, D], bf16)
            nc.sync.dma_start(out=wc0[:], in_=wcat0[:])
            wc1 = cpool.tile([P, R * KC, D], bf16)
            nc.sync.dma_start(out=wc1[:], in_=wcat1[:])

            tsT_sb = cpool.tile([P, SPAN_K // P, B], f32)
            nc.sync.dma_start(out=tsT_sb[:], in_=tsT[:])
            b1s_sb = cpool.tile([P, SPAN_SL // P], f32)
            nc.sync.dma_start(out=b1s_sb[:], in_=b1s[:])
            b2f_sb = cpool.tile([P, KC], f32)
            nc.sync.dma_start(out=b2f_sb[:], in_=b2f[:])
            fpb1_sb = cpool.tile([P, KC], f32)
            nc.sync.dma_start(out=fpb1_sb[:], in_=fpb1[:])
            fpb2_sb = cpool.tile([P, KC], f32)
            nc.sync.dma_start(out=fpb2_sb[:], in_=fpb2[:])

            # zero pad rows of x2 (row ZROW used to mask non-owned gathers)
            zt = cpool.tile([P, D], bf16)
            nc.vector.memset(zt[:], 0.0)
            nc.sync.dma_start(out=x2_sl[NLOC:X2ROWS, :], in_=zt[:])

            # ---------- span MLP (K-sharded; independent of GNN)
            h1T_sb = pool.tile([P, SPAN_SL // P, B], f32, tag="h1T")
            for mc in range(SPAN_SL // P):
                ph = psS.tile([P, B], f32, tag="sp")
                for kc in range(SPAN_K // P):
                    w1t = pool.tile([P, P], f32, tag="w1t")
                    nc.sync.dma_start(out=w1t[:], in_=w1s[:, kc, mc, :])
                    nc.tensor.matmul(out=ph[:], lhsT=w1t[:], rhs=tsT_sb[:, kc, :],
                                     start=(kc == 0), stop=(kc == SPAN_K // P - 1))
                nc.scalar.activation(out=h1T_sb[:, mc, :], in_=ph[:], func=AF.Relu,
                                     bias=b1s_sb[:, mc:mc + 1])
            tnp_sb = pool.tile([P, KC, B], f32, tag="tnp")
            for mc in range(KC):
                ph = psS.tile([P, B], f32, tag="sp")
                for kc in range(SPAN_SL // P):
                    w2t = pool.tile([P, P], f32, tag="w1t")
                    nc.sync.dma_start(out=w2t[:], in_=w2s[:, kc, mc, :])
                    nc.tensor.matmul(out=ph[:], lhsT=w2t[:],
                                     rhs=h1T_sb[:, kc, :],
                                     start=(kc == 0), stop=(kc == SPAN_SL // P - 1))
                nc.vector.tensor_copy(out=tnp_sb[:, mc, :], in_=ph[:])
            nc.sync.dma_start(out=tn_part[:], in_=tnp_sb[:])
            # dispatched first on the CC stream: absorbs the collective
            # cold-start in parallel with layer 0 (consumers come after L1).
            nc.gpsimd.collective_compute(
                "AllReduce", ALU.add, replica_groups=groups,
                ins=[tn_part[:]], outs=[tn_red[:]])

            # ---------- GNN layers
            for layer in range(2):
                wc = wc0 if layer == 0 else wc1
                if layer == 1:
                    nc.gpsimd.collective_compute(
                        "AllGather", ALU.bypass, replica_groups=groups,
                        ins=[x1_slice[:]], outs=[x_full[:]])

                for b in range(BPC):
                    gT_sb = pool.tile([P, R * KC, P], bf16, tag="gT")
                    for r in range(R):
                        bat = b * R + r
                        t0 = bat * tpb
                        if layer == 0:
                            xg = gpool.tile([P, tpb, D], bf16, tag="xg0")
                            nc.sync.dma_start(out=xg[:],
                                              in_=xg0[:, t0:t0 + tpb, :])

                            def tsl(t, xg=xg):
                                return xg[:, t, :]
                        else:
                            i0 = bat * (cap // 16)
                            mb0 = gpool.tile([P, h0, D], bf16, tag="mbufa")
                            mb1 = gpool.tile([P, h1, D], bf16, tag="mbufb")
                            nc.gpsimd.dma_gather(
                                out_ap=mb0[:], in_ap=x_full[:],
                                idxs_ap=idx_sb[:, i0:i0 + h0 * P // 16],
                                num_idxs=h0 * P, num_idxs_reg=h0 * P,
                                elem_size=D, elem_step=D)
                            nc.gpsimd.dma_gather(
                                out_ap=mb1[:], in_ap=x_full[:],
                                idxs_ap=idx_sb[:, i0 + h0 * P // 16:i0 + cap // 16],
                                num_idxs=h1 * P, num_idxs_reg=h1 * P,
                                elem_size=D, elem_step=D)

                            def tsl(t, a=mb0, b=mb1):
                                if t < h0:
                                    return a[:, t, :]
                                return b[:, t - h0, :]
                        s_sb = pool.tile([P, cap], bf16, tag="s")
                        rows_b = rows_sb[:, t0:t0 + tpb].unsqueeze(2).broadcast_to([P, tpb, P])
                        nc.vector.tensor_tensor(
                            out=s_sb[:].rearrange("p (t w) -> p t w", w=P),
                            in0=rows_b,
                            in1=iota_sb[:].rearrange("p (t w) -> p t w", w=P),
                            op=ALU.is_equal)
                        vals_b = vals_sb[:, t0:t0 + tpb].unsqueeze(2).broadcast_to([P, tpb, P])
                        nc.vector.tensor_tensor(
                            out=s_sb[:].rearrange("p (t w) -> p t w", w=P),
                            in0=s_sb[:].rearrange("p (t w) -> p t w", w=P),
                            in1=vals_b,
                            op=ALU.mult)
                        pg = psA.tile([P, D], f32, tag="pg")
                        for t in range(tpb):
                            nc.tensor.matmul(
                                out=pg[:],
                                lhsT=s_sb[:, t * P:(t + 1) * P],
                                rhs=tsl(t),
                                start=(t == 0), stop=(t == tpb - 1))
                        g_sb = pool.tile([P, D], bf16, tag="g")
                        nc.scalar.activation(out=g_sb[:], in_=pg[:], func=AF.Copy)
                        ptr = psA.tile([P, D], bf16, tag="ptr")
                        for c in range(KC):
                            nc.tensor.transpose(out=ptr[:, c * P:(c + 1) * P],
                                                in_=g_sb[:, c * P:(c + 1) * P],
                                                identity=ident[:])
                        nc.scalar.activation(
                            out=gT_sb[:, r * KC:(r + 1) * KC, :],
                            in_=ptr[:].rearrange("p (c w) -> p c w", w=P),
                            func=AF.Copy)
                    po = psB.tile([P, D], f32, tag="po")
                    for j in range(R * KC):
                        nc.tensor.matmul(out=po[:], lhsT=gT_sb[:, j, :],
                                         rhs=wc[:, j, :],
                                         start=(j == 0), stop=(j == R * KC - 1))
                    xo = pool.tile([P, D], bf16, tag="xo")
                    nc.scalar.activation(out=xo[:], in_=po[:], func=AF.Tanh)
                    if layer == 0:
                        nc.sync.dma_start(out=x1_slice[b * P:(b + 1) * P, :], in_=xo[:])
                    else:
                        nc.sync.dma_start(out=x2_sl[b * P:(b + 1) * P, :], in_=xo[:])

            # ---------- span MLP tail (consumes tn_red; placed after the GNN
            # loops so no engine queue stalls on the AllReduce mid-layer-0)
            tnT_sb = pool.tile([P, KC, B], f32, tag="tnT")
            tnr_sb = pool.tile([P, KC, B], f32, tag="tnr")
            nc.sync.dma_start(out=tnr_sb[:], in_=tn_red[:])
            for mc in range(KC):
                nc.vector.tensor_scalar_add(out=tnT_sb[:, mc, :], in0=tnr_sb[:, mc, :],
                                            scalar1=b2f_sb[:, mc:mc + 1])
            h2T_sb = pool.tile([P, KC, B], f32, tag="h2T")
            for mc in range(KC):
                ph = psS.tile([P, B], f32, tag="sp")
                for kc in range(KC):
                    fp1t = pool.tile([P, P], f32, tag="w1t")
                    nc.sync.dma_start(out=fp1t[:], in_=fpw1[:, kc, mc, :])
                    nc.tensor.matmul(out=ph[:], lhsT=fp1t[:],
                                     rhs=tnT_sb[:, kc, :],
                                     start=(kc == 0), stop=(kc == KC - 1))
                nc.scalar.activation(out=h2T_sb[:, mc, :], in_=ph[:], func=AF.Relu,
                                     bias=fpb1_sb[:, mc:mc + 1])
            qTb_sb = pool.tile([P, KC, B], bf16, tag="qTb")
            for mc in range(KC):
                ph = psS.tile([P, B], f32, tag="sp")
                for kc in range(KC):
                    fp2t = pool.tile([P, P], f32, tag="w1t")
                    nc.sync.dma_start(out=fp2t[:], in_=fpw2[:, kc, mc, :])
                    nc.tensor.matmul(out=ph[:], lhsT=fp2t[:],
                                     rhs=h2T_sb[:, kc, :],
                                     start=(kc == 0), stop=(kc == KC - 1))
                nc.scalar.activation(out=qTb_sb[:, mc, :], in_=ph[:], func=AF.Tanh,
                                     bias=fpb2_sb[:, mc:mc + 1])

            # ---------- final (localized): pred cols for OWN frames + own gold
            frT0 = cpool.tile([P, KC, FRH], bf16, tag="frT0")
            frT1 = cpool.tile([P, KC, FRH], bf16, tag="frT1")
            frT = [frT0, frT1]
            for h in range(2):
                nc.gpsimd.dma_gather(
                    out_ap=frT[h][:],
                    in_ap=x2_sl[:],
                    idxs_ap=fridx_sb[:, h * (FRH // 16):(h + 1) * (FRH // 16)],
                    num_idxs=FRH, num_idxs_reg=FRH,
                    elem_size=D, transpose=True)
            pg_sb = cpool.tile([B, FRAMES + D], f32, tag="pgsb")
            chunks = [(0, 0, 512), (0, 512, 128), (1, 0, 512), (1, 512, 48)]
            lo = 0
            for (h, off, w) in chunks:
                pp = psB.tile([B, w], f32, tag="po")
                for c in range(KC):
                    nc.tensor.matmul(out=pp[:], lhsT=qTb_sb[:, c, :],
                                     rhs=frT[h][:, c, off:off + w],
                                     start=(c == 0), stop=(c == KC - 1))
                nc.vector.tensor_copy(out=pg_sb[:, lo:lo + w], in_=pp[:])
                lo += w
            assert lo == FRAMES
            gold_sb = pool.tile([P, 1, D], bf16, tag="goldb")
            nc.gpsimd.dma_gather(
                out_ap=gold_sb[:], in_ap=x2_sl[:], idxs_ap=goldidx_sb[:],
                num_idxs=P, num_idxs_reg=P, elem_size=D, elem_step=D)
            nc.vector.tensor_copy(out=pg_sb[:, FRAMES:], in_=gold_sb[:B, 0, :])
            nc.sync.dma_start(out=pg_part[:], in_=pg_sb[:])
            nc.gpsimd.collective_compute(
                "AllReduce", ALU.add, replica_groups=groups,
                ins=[pg_part[:]], outs=[pg_red[:]])
            nc.sync.dma_start(out=out[:], in_=pg_red[:])

    nc.compile()
    return nc


def get_program(tpb):
    if tpb not in _nc_cache:
        _nc_cache[tpb] = build_program(tpb)
    return _nc_cache[tpb]


# ---------------------------------------------------------------- host prep


def _bf16():
    import ml_dtypes
    return ml_dtypes.bfloat16


def _wrap_idx16(flat):
    a = np.asarray(flat, np.int16).reshape(-1, 16).T  # [16, n/16]
    return np.tile(a, (8, 1)).copy()


def _segpos(bin_id, i):
    """Global x_full position of slot i of bin `bin_id` (bin-order layout,
    matching the single AllGather concatenation)."""
    return bin_id * P + i


def _pack_nodes(adj_rows):
    """Balanced 5-dim bin packing: assign NPAD nodes to NBLK bins of P nodes
    so that per-(bin, relation) in-edge counts are as flat as possible.
    Returns (perm, pos_of, bin_of, row_of, cap)."""
    deg = np.zeros((NPAD, R), np.int64)
    for r in range(R):
        np.add.at(deg[:, r], adj_rows[r], 1)
    order = np.argsort(-deg.sum(1), kind="stable")
    bin_load = np.zeros((NBLK, R), np.int64)
    bin_cnt = np.zeros(NBLK, np.int64)
    assign = np.empty(NPAD, np.int64)
    for n in order:
        d = deg[n]
        cand = np.where(bin_cnt < P)[0]
        newmax = (bin_load[cand] + d).max(1)
        best = cand[np.lexsort((bin_load[cand].sum(1), newmax))[0]]
        assign[n] = best
        bin_load[best] += d
        bin_cnt[best] += 1
    maxload = int(bin_load.max())
    cap = ((maxload + P - 1) // P) * P
    cap = max(cap, 2 * P)
    perm = np.empty(NPAD, np.int64)
    pos_of = np.empty(NPAD, np.int64)
    bin_of = np.empty(NPAD, np.int64)
    row_of = np.empty(NPAD, np.int64)
    for bin_id in range(NBLK):
        nodes = np.where(assign == bin_id)[0]
        q = _segpos(bin_id, np.arange(P))
        perm[q] = nodes
        pos_of[nodes] = q
        bin_of[q] = bin_id
        row_of[q] = np.arange(P)
    return perm, pos_of, bin_of, row_of, cap


def preprocess(inputs):
    bf = _bf16()
    ts = np.ascontiguousarray(np.asarray(inputs["target_span"], np.float32))
    frame_emb = np.asarray(inputs["frame_emb"], np.float32)
    role_emb = np.asarray(inputs["role_emb"], np.float32)
    rel_W0 = np.asarray(inputs["rel_W0"], np.float32)
    rel_W1 = np.asarray(inputs["rel_W1"], np.float32)
    span_W1 = np.asarray(inputs["span_W1"], np.float32)
    span_b1 = np.asarray(inputs["span_b1"], np.float32)
    span_W2 = np.asarray(inputs["span_W2"], np.float32)
    span_b2 = np.asarray(inputs["span_b2"], np.float32)
    fp_W1 = np.asarray(inputs["fp_W1"], np.float32)
    fp_b1 = np.asarray(inputs["fp_b1"], np.float32)
    fp_W2 = np.asarray(inputs["fp_W2"], np.float32)
    fp_b2 = np.asarray(inputs["fp_b2"], np.float32)
    adj_vals = np.asarray(inputs["adj_vals"], np.float32)
    fe_ids = np.asarray(inputs["fe_ids"]).astype(np.int64)
    adj_rows = np.asarray(inputs["adj_rows"]).astype(np.int64)
    adj_cols = np.asarray(inputs["adj_cols"]).astype(np.int64)
    gold_frame_id = np.asarray(inputs["gold_frame_id"]).astype(np.int64)
    frame_list = np.asarray(inputs["frame_list"]).astype(np.int64)

    perm, pos_of, bin_of, row_of, cap = _pack_nodes(
        [adj_rows[r] for r in range(R)])
    tpb = cap // P
    core_of = bin_of // BPC
    loc_of = (bin_of % BPC) * P + row_of     # local x2 row per position

    g_idx = np.zeros((R, NBLK, cap), np.int64)
    g_rows = np.zeros((R, NBLK, cap), np.float32)
    g_vals = np.zeros((R, NBLK, cap), np.float32)
    for r in range(R):
        pos_r = pos_of[adj_rows[r]]
        blk = bin_of[pos_r]
        order = np.argsort(blk, kind="stable")
        blk_s = blk[order]
        counts = np.bincount(blk_s, minlength=NBLK)
        assert counts.max() <= cap
        starts = np.zeros(NBLK, np.int64)
        starts[1:] = np.cumsum(counts)[:-1]
        rank = np.arange(E) - starts[blk_s]
        dest = blk_s * cap + rank
        g_idx[r].flat[dest] = pos_of[adj_cols[r][order]]
        g_rows[r].flat[dest] = row_of[pos_r[order]].astype(np.float32)
        g_vals[r].flat[dest] = adj_vals[r][order]

    emb_cat = np.concatenate([frame_emb[:FRAMES], role_emb], axis=0)
    assert emb_cat.shape == (N, D)

    node_at = perm
    src_all = np.where(node_at < FRAMES, node_at,
                       np.where(node_at < N,
                                FRAMES + fe_ids[np.clip(node_at - FRAMES, 0, NROLE - 1)],
                                0))
    emb_src = emb_cat[src_all].astype(bf)   # [NPAD, D] layer-0 x by position

    wcat0 = rel_W0.reshape(R, KC, P, D).transpose(2, 0, 1, 3).reshape(P, R * KC, D)
    wcat1 = rel_W1.reshape(R, KC, P, D).transpose(2, 0, 1, 3).reshape(P, R * KC, D)
    tsT = ts.T.reshape(SPAN_K // P, P, B).transpose(1, 0, 2)
    fpw1 = fp_W1.reshape(KC, P, KC, P).transpose(1, 0, 2, 3)
    fpw2 = fp_W2.reshape(KC, P, KC, P).transpose(1, 0, 2, 3)
    b2f = span_b2.reshape(KC, P).T
    fpb1v = fp_b1.reshape(KC, P).T
    fpb2v = fp_b2.reshape(KC, P).T

    iota = np.tile(np.arange(P, dtype=np.float32), tpb)[None, :].repeat(P, axis=0)

    gold_label = frame_list[np.arange(B), gold_frame_id]
    gold_pos = pos_of[gold_label]
    fr_pos = np.zeros(FR_PAD, np.int64)
    fr_pos[:FRAMES] = pos_of[np.arange(FRAMES)]
    fr_pos[FRAMES:] = -1

    in_maps = []
    for k in range(NCORE):
        blo, bhi = k * BPC, (k + 1) * BPC
        ci = g_idx[:, blo:bhi].transpose(1, 0, 2)       # [BPC, R, cap]
        cr = g_rows[:, blo:bhi].transpose(1, 0, 2)
        cv = g_vals[:, blo:bhi].transpose(1, 0, 2)
        flat_idx = ci.reshape(-1)
        gidx_c = _wrap_idx16(flat_idx)
        xg0_c = emb_src[flat_idx].reshape(-1, P, D).transpose(1, 0, 2)
        rows_c = cr.reshape(-1, P).T.astype(bf)
        vals_c = cv.reshape(-1, P).T.astype(bf)
        sl = slice(k * SPAN_SL, (k + 1) * SPAN_SL)
        w1slice = span_W1[:, sl]
        w1s = w1slice.reshape(SPAN_K // P, P, SPAN_SL // P, P).transpose(1, 0, 2, 3)
        b1sv = span_b1[sl].reshape(SPAN_SL // P, P).T
        w2slice = span_W2[sl, :]
        w2s = w2slice.reshape(SPAN_SL // P, P, KC, P).transpose(1, 0, 2, 3)

        frp = np.clip(fr_pos, 0, NPAD - 1)
        own_fr = (fr_pos >= 0) & (core_of[frp] == k)
        fr_local = np.where(own_fr, loc_of[frp], ZROW)
        own_gold = core_of[gold_pos] == k
        gold_local = np.where(own_gold, loc_of[gold_pos], ZROW)

        in_maps.append(dict(
            gidx=np.ascontiguousarray(gidx_c),
            xg0=np.ascontiguousarray(xg0_c),
            rows_sh=np.ascontiguousarray(rows_c),
            vals_in=np.ascontiguousarray(vals_c),
            iota_in=iota.astype(bf),
            wcat0=np.ascontiguousarray(wcat0).astype(bf),
            wcat1=np.ascontiguousarray(wcat1).astype(bf),
            tsT=np.ascontiguousarray(tsT),
            w1s=np.ascontiguousarray(w1s),
            b1s=np.ascontiguousarray(b1sv),
            w2s=np.ascontiguousarray(w2s),
            b2f=np.ascontiguousarray(b2f),
            fpw1=np.ascontiguousarray(fpw1),
            fpb1=np.ascontiguousarray(fpb1v),
            fpw2=np.ascontiguousarray(fpw2),
            fpb2=np.ascontiguousarray(fpb2v),
            fridx=_wrap_idx16(fr_local),
            goldidx=_wrap_idx16(np.concatenate(
                [gold_local, np.full(P - B, ZROW, np.int64)])),
        ))
    return in_maps, tpb


def _maybe_enable_trace():
    import types
    import antenv
    if getattr(antenv, "axon_hooks", None) is not None:
        return
    mod = types.ModuleType("antenv.axon_hooks")
    state = {}
    mod.set_axon_ntff_profile_hook = lambda h: state.__setitem__("h", h)
    mod.get_axon_ntff_profile_hook = lambda: state.get("h")
    sys.modules["antenv.axon_hooks"] = mod
    antenv.axon_hooks = mod
    from trn_agent_boot.trn_boot import _ntff_profile_via_ctypes
    mod.set_axon_ntff_profile_hook(_ntff_profile_via_ctypes("/opt/axon/libaxon_pjrt.so"))


def kernel(**inputs):
    from concourse.bass_utils import run_bass_kernel_spmd

    trace = os.environ.get("KERNEL_TRACE", "0") == "1"
    if trace:
        _maybe_enable_trace()

    in_maps, tpb = preprocess(inputs)
    nc = get_program(tpb)
    kw = {}
    if trace:
        import tempfile
        kw = dict(trace=True, tmpdir=tempfile.mkdtemp(prefix="ktrace_"))
    res = run_bass_kernel_spmd(nc, in_maps, list(range(NCORE)), **kw)
    if trace:
        kernel.last_exec_time_ns = res.exec_time_ns
    return np.asarray(res.results[0]["out"], np.float32)


kernel.last_exec_time_ns = None
